# revision 1
# baseline (speedup 1.0000x reference)
"""Multi-head attention forward (B=8, S=1024, H=16, D=64) on 8 TRN2 NeuronCores.

Sharding: pure data-parallel over batch — core b computes batch element b
end-to-end (QKV projections + 16-head attention). Zero collectives.

Per-core dataflow (bf16 matmuls, fp32 PSUM accumulation), scheduled around
DMA arrival and the ScalarE exp cadence:
  - weights for pairs 0-1 load as fine per-column slices on the SWDGE queue
    (interleaved with x_to's lo half, which loads bf16-cast on the same
    queue), so the first scores wait only on x_from + ~1MB of weights; the
    remaining weight columns stream as per-kt row-block slices (3KB rows)
    while the pair loop runs and land just in time for pairs 2-7.
  - x_from + x_to's hi half stream on the HWDGE queue in 256-row chunks;
    their PE transposes interleave with pair-0's projection chains, and
    Q/K/V^T tiles are split into 512-column half-tiles so dependencies
    release as each half of x lands.
  - biases load as [8,128] rows (8 fat descriptors) + one PE transpose each.
  - the pair loop software-pipelines back(p-1) work (ctx, out-transpose,
    normalize, store) into the exp-cadence gaps of front(p), with the next
    pair's Q/K chains as tail filler, so the PE stays dense across pair
    boundaries and ctx never waits on the previous pair's last exp.
  - scores keep both heads in one 4-bank PSUM tile; the K=64 matmuls stay
    adjacent in the PE stream and pack onto disjoint row-groups, running
    concurrently.
  - V' strips build with one [128,128] PE transpose per s-tile (both heads);
    softmax denominators ride the ones-column of V' and are reciprocal'd
    8-at-a-time after the output transpose.
"""

import numpy as np
from contextlib import ExitStack

import concourse.bass as bass
import concourse.mybir as mybir
import concourse.tile as tile
from concourse import bacc
from concourse.masks import make_identity
from concourse.bass_utils import run_bass_kernel_spmd

B, S, H, D = 8, 1024, 16, 64
W = H * D  # 1024
P = 128
N_CORES = 8
F32 = mybir.dt.float32
BF16 = mybir.dt.bfloat16
AF = mybir.ActivationFunctionType
ALU = mybir.AluOpType

ST = S // P   # 8 s-tiles
KT_ = W // P  # 8 contraction tiles
IH = 2        # 512-wide halves of the moving dim
HD1 = D + 1   # 65: V' width per head
NP = H // 2   # 8 head pairs
VW = 2 * HD1  # 130: V' slot width per s-tile (two heads + ones cols)
NFINE = 2     # pairs covered by fine column-sliced weight loads
BW = W - NFINE * P  # 768: bulk column width


def build_kernel():
    nc = bacc.Bacc(trn_type="TRN2", target_bir_lowering=False, debug=False,
                   num_devices=N_CORES)

    xf_ext = nc.dram_tensor("from_tensor", [S, W], F32, kind="ExternalInput").ap()
    xt_ext = nc.dram_tensor("to_tensor", [S, W], F32, kind="ExternalInput").ap()
    wq_ext = nc.dram_tensor("Wq", [W, W], F32, kind="ExternalInput").ap()
    bq_ext = nc.dram_tensor("bq", [W], F32, kind="ExternalInput").ap()
    wk_ext = nc.dram_tensor("Wk", [W, W], F32, kind="ExternalInput").ap()
    bk_ext = nc.dram_tensor("bk", [W], F32, kind="ExternalInput").ap()
    wv_ext = nc.dram_tensor("Wv", [W, W], F32, kind="ExternalInput").ap()
    bv_ext = nc.dram_tensor("bv", [W], F32, kind="ExternalInput").ap()
    out_ext = nc.dram_tensor("out", [S, W], F32, kind="ExternalOutput").ap()
    w_exts = {"q": wq_ext, "k": wk_ext, "v": wv_ext}

    with tile.TileContext(nc) as tc, ExitStack() as top:
        const = top.enter_context(tc.tile_pool(name="const", bufs=1))
        big = top.enter_context(tc.tile_pool(name="big", bufs=1))
        xr_pool = top.enter_context(tc.tile_pool(name="xr", bufs=2))
        xc_pool = top.enter_context(tc.tile_pool(name="xc", bufs=2))
        pp_pool = top.enter_context(tc.tile_pool(name="pp", bufs=1))
        et_pool = top.enter_context(tc.tile_pool(name="et", bufs=16))
        sm_pool = top.enter_context(tc.tile_pool(name="sm", bufs=1))
        ps_proj = top.enter_context(
            tc.tile_pool(name="ps_proj", bufs=2, space="PSUM"))
        ps_s = top.enter_context(
            tc.tile_pool(name="ps_s", bufs=1, space="PSUM"))
        ps_c = top.enter_context(
            tc.tile_pool(name="ps_c", bufs=2, space="PSUM"))

        # ---- identity matrices (gpsimd queue, before weight descriptors) ----
        ident = const.tile([P, P], BF16, tag="ident")
        make_identity(nc, ident[:])
        idf32 = const.tile([8, 8], F32, tag="idf32")
        make_identity(nc, idf32[:])

        # ---- weight tiles ----
        wfine = {}
        for nm in ("q", "k", "v"):
            for mt in range(NFINE):
                wfine[nm, mt] = big.tile([P, KT_ * P], BF16,
                                         tag=f"wf{nm}{mt}", name=f"wf{nm}{mt}")
        wbulk = {}
        for nm in ("q", "k", "v"):
            for kt in range(KT_):
                wbulk[nm, kt] = big.tile([P, BW], BF16, tag=f"wb{nm}{kt}",
                                         name=f"wb{nm}{kt}")

        def w_slice(nm, mt, kt):
            if mt < NFINE:
                return wfine[nm, mt][:, kt * P:(kt + 1) * P]
            return wbulk[nm, kt][:, (mt - NFINE) * P:(mt - NFINE + 1) * P]

        # ---- DMA issue: sync (HWDGE) queue ----
        brow = const.tile([8, 3 * P], F32, tag="brow")
        for i, b_ext in enumerate((bq_ext, bk_ext, bv_ext)):
            nc.sync.dma_start(brow[:, i * P:(i + 1) * P],
                              b_ext.rearrange("(t p) -> t p", p=P))

        def x_chunk_load(x_ext, ch, name):
            xr = xr_pool.tile([P, 2 * W], F32, tag="xr", name=name)
            nc.sync.dma_start(
                xr.rearrange("p (t f) -> p t f", f=W),
                x_ext.rearrange("(t p) f -> p t f", p=P)[
                    :, ch * 2:(ch + 1) * 2, :])
            return xr

        # ---- gpsimd (SWDGE) queue: xt-lo chunks (cast), fine c1, bulk ----
        def xt_chunk_load_sw(ch, name):
            # dedicated tiles: ring slots would add a WAR pointing forward in
            # the PE stream (deadlock) since these are consumed mid-pair-0
            xc = big.tile([P, 2 * W], BF16, tag=name, name=name)
            nc.gpsimd.dma_start(
                xc.rearrange("p (t f) -> p t f", f=W),
                xt_ext.rearrange("(t p) f -> p t f", p=P)[
                    :, ch * 2:(ch + 1) * 2, :])
            return xc

        def load_w_fine_sw(nm, mt):
            nc.gpsimd.dma_start(
                wfine[nm, mt].rearrange("p (t c) -> p t c", c=P),
                w_exts[nm].rearrange("(t p) f -> p t f", p=P)[
                    :, :, mt * P:(mt + 1) * P])

        load_w_fine_sw("q", 0)
        xt_lo = [xt_chunk_load_sw(0, "xtlo0"), xt_chunk_load_sw(1, "xtlo1")]
        load_w_fine_sw("k", 0)
        load_w_fine_sw("v", 0)
        load_w_fine_sw("q", 1)
        load_w_fine_sw("k", 1)
        load_w_fine_sw("v", 1)
        for nm in ("q", "k", "v"):
            for kt in range(KT_):
                nc.gpsimd.dma_start(
                    wbulk[nm, kt][:],
                    w_exts[nm].rearrange("(t p) f -> p t f", p=P)[
                        :, kt, NFINE * P:])

        # ---- x^T half tiles ----
        xTf = [big.tile([P, KT_ * 512], BF16, tag=f"xTf{h}", name=f"xTf{h}")
               for h in range(2)]
        xTt = [big.tile([P, KT_ * 512], BF16, tag=f"xTt{h}", name=f"xTt{h}")
               for h in range(2)]

        # ---- bias transpose: [8,128] rows -> [128,8] columns ----
        b_sb = const.tile([P, 24], F32, tag="b_sb")
        bps = ps_proj.tile([P, 24], F32, tag="proj", name="bps")
        for i in range(3):
            nc.tensor.transpose(bps[:, i * 8:(i + 1) * 8],
                                brow[:, i * P:(i + 1) * P], idf32[:])
        nc.vector.tensor_copy(b_sb[:], bps[:])

        # ---- x chunk processing ----
        def x_chunk_cast(xr, name):
            xc = xc_pool.tile([P, 2 * W], BF16, tag="xc", name=name)
            nc.vector.tensor_copy(xc[:], xr[:])
            return xc

        def x_chunk_transpose(xc, xT_half, sub):
            for sl in range(2):
                pt = ps_proj.tile([P, KT_ * P], BF16, tag="proj", name="ptx")
                for wt in range(KT_):
                    nc.tensor.transpose(
                        pt[:, wt * P:(wt + 1) * P],
                        xc[:, sl * W + wt * P: sl * W + wt * P + P],
                        ident[:])
                nc.vector.tensor_copy(
                    xT_half.rearrange("p (w s) -> p w s", s=512)[
                        :, :, sub * 256 + sl * P: sub * 256 + (sl + 1) * P],
                    pt.rearrange("p (w c) -> p w c", c=P))

        # ---- pair-loop building blocks ----
        def proj_half(dst_half, nm, xT, mt, ih):
            ps = ps_proj.tile([P, 512], F32, tag="proj", name="pp")
            for kt in range(KT_):
                nc.tensor.matmul(
                    ps[:],
                    lhsT=w_slice(nm, mt, kt),
                    rhs=xT[ih][:, kt * 512:(kt + 1) * 512],
                    start=(kt == 0), stop=(kt == KT_ - 1))
            bof = {"q": 0, "k": 8, "v": 16}[nm]
            nc.vector.tensor_scalar_add(
                dst_half[:], ps[:], b_sb[:, bof + mt:bof + mt + 1])

        def scores_jt(QT, KT2, jt, Et):
            # both heads share ONE 4-bank PSUM tile; the K=64 matmuls stay
            # adjacent in the PE stream and pack onto disjoint row-groups
            # (0-63 / 64-127), running concurrently.
            kh = KT2[jt // 4]
            pss = ps_s.tile([P, 2 * S], F32, tag="pss", name="pss")
            for ih in range(IH):
                for hh in range(2):
                    ho = hh * D
                    nc.tensor.matmul(
                        pss[:, hh * S + ih * 512: hh * S + (ih + 1) * 512],
                        lhsT=kh[ho:ho + D, (jt % 4) * P:(jt % 4) * P + P],
                        rhs=QT[ih][ho:ho + D, :],
                        start=True, stop=True)
            et = et_pool.tile([P, 2 * S], BF16, tag="et", name="et")
            nc.scalar.activation(et[:], pss[:], AF.Exp, scale=0.125)
            Et[jt] = et

        def vprime_strips(VT2, Vp, jts):
            for jt in jts:
                pv = ps_proj.tile([P, P], BF16, tag="proj", name="pv")
                nc.tensor.transpose(
                    pv[:], VT2[jt // 4][:, (jt % 4) * P:(jt % 4 + 1) * P],
                    ident[:])
                nc.vector.tensor_copy(
                    Vp.rearrange("p (j g c) -> p j g c", g=2, c=HD1)[
                        :, jt, :, 0:D],
                    pv.rearrange("p (g c) -> p g c", c=D))

        def vprime_ones(Vp):
            nc.vector.memset(
                Vp.rearrange("p (j g c) -> p j g c", g=2, c=HD1)[
                    :, :, :, D:HD1], 1.0)

        def ctx_chunk(prev, hh, ih):
            Vp, Et = prev["Vp"], prev["Et"]
            pc = ps_c.tile([HD1, 512], F32, tag="pc", name="pc")
            for jt in range(ST):
                nc.tensor.matmul(
                    pc[:],
                    lhsT=Vp[:, jt * VW + hh * HD1: jt * VW + (hh + 1) * HD1],
                    rhs=Et[jt][:, hh * S + ih * 512: hh * S + (ih + 1) * 512],
                    start=(jt == 0), stop=(jt == ST - 1))
            nc.vector.tensor_copy(
                prev["ctxb"][hh][:, ih * 512:(ih + 1) * 512], pc[:])

        def outT_norm(prev, hh):
            ctxb = prev["ctxb"][hh]
            out_p = prev["out_p"]
            # stride 66 keeps each bf16 PSUM slice 4-byte aligned
            po = ps_proj.tile([P, ST * 66], BF16, tag="proj", name="po")
            for it in range(ST):
                nc.tensor.transpose(
                    po[:, it * 66: it * 66 + HD1],
                    ctxb[:, it * P:(it + 1) * P],
                    ident[0:HD1, 0:HD1])
            rinv = sm_pool.tile([P, 8], F32, tag="rinv", bufs=3, name="rinv")
            nc.vector.reciprocal(
                rinv.rearrange("p (a b) -> p a b", b=1),
                po.rearrange("p (it c) -> p it c", c=66)[:, :, D:HD1])
            for it in range(ST):
                nc.vector.tensor_scalar_mul(
                    out_p[:, it * P + hh * D: it * P + hh * D + D],
                    po[:, it * 66: it * 66 + D], rinv[:, it:it + 1])

        def out_dma(prev):
            nc.sync.dma_start(
                out_ext.rearrange("(t p) (g c) -> p t g c", p=P, c=P)[
                    :, :, prev["mt"], :],
                prev["out_p"].rearrange("p (t c) -> p t c", c=P))

        def pair_tiles(mt):
            QT = [pp_pool.tile([P, 512], BF16, tag=f"qt{h}", bufs=2,
                               name="QT") for h in range(2)]
            KT2 = [pp_pool.tile([P, 512], BF16, tag=f"kt{h}", bufs=2,
                                name="KT") for h in range(2)]
            VT2 = [pp_pool.tile([P, 512], BF16, tag=f"vt{h}", bufs=2,
                                name="VT") for h in range(2)]
            Vp = pp_pool.tile([P, ST * VW], BF16, tag="vp", bufs=2, name="Vp")
            ctxb0 = sm_pool.tile([HD1, S], BF16, tag="ctxb", bufs=2,
                                 name="ctxb0")
            ctxb1 = sm_pool.tile([HD1, S], BF16, tag="ctxb", bufs=2,
                                 name="ctxb1")
            out_p = pp_pool.tile([P, ST * P], F32, tag="outp", bufs=2,
                                 name="out_p")
            return {"mt": mt, "QT": QT, "KT2": KT2, "VT2": VT2, "Vp": Vp,
                    "ctxb": (ctxb0, ctxb1), "out_p": out_p, "Et": {}}

        # ---- pair 0 + phase 0, interleaved ----
        p0 = pair_tiles(0)
        xf_r = [x_chunk_load(xf_ext, ch, f"xrf{ch}") for ch in range(2)]
        x_chunk_transpose(xt_lo[0], xTt[0], 0)
        xc0 = x_chunk_cast(xf_r[0], "xcf0")
        x_chunk_transpose(xc0, xTf[0], 0)
        xf_r.append(x_chunk_load(xf_ext, 2, "xrf2"))
        x_chunk_transpose(xt_lo[1], xTt[0], 1)
        xc1 = x_chunk_cast(xf_r[1], "xcf1")
        x_chunk_transpose(xc1, xTf[0], 1)
        xf_r.append(x_chunk_load(xf_ext, 3, "xrf3"))
        proj_half(p0["QT"][0], "q", xTf, 0, 0)
        proj_half(p0["KT2"][0], "k", xTt, 0, 0)
        xc2 = x_chunk_cast(xf_r[2], "xcf2")
        x_chunk_transpose(xc2, xTf[1], 0)
        xt_r = [x_chunk_load(xt_ext, 2, "xrt2")]
        xc3 = x_chunk_cast(xf_r[3], "xcf3")
        x_chunk_transpose(xc3, xTf[1], 1)
        xt_r.append(x_chunk_load(xt_ext, 3, "xrt3"))
        proj_half(p0["QT"][1], "q", xTf, 0, 1)
        scores_jt(p0["QT"], p0["KT2"], 0, p0["Et"])
        scores_jt(p0["QT"], p0["KT2"], 1, p0["Et"])
        xct2 = x_chunk_cast(xt_r[0], "xct2")
        x_chunk_transpose(xct2, xTt[1], 0)
        scores_jt(p0["QT"], p0["KT2"], 2, p0["Et"])
        xct3 = x_chunk_cast(xt_r[1], "xct3")
        x_chunk_transpose(xct3, xTt[1], 1)
        scores_jt(p0["QT"], p0["KT2"], 3, p0["Et"])
        proj_half(p0["KT2"][1], "k", xTt, 0, 1)
        scores_jt(p0["QT"], p0["KT2"], 4, p0["Et"])
        proj_half(p0["VT2"][0], "v", xTt, 0, 0)
        scores_jt(p0["QT"], p0["KT2"], 5, p0["Et"])
        proj_half(p0["VT2"][1], "v", xTt, 0, 1)
        scores_jt(p0["QT"], p0["KT2"], 6, p0["Et"])
        vprime_strips(p0["VT2"], p0["Vp"], range(0, 4))
        scores_jt(p0["QT"], p0["KT2"], 7, p0["Et"])
        vprime_strips(p0["VT2"], p0["Vp"], range(4, 8))
        vprime_ones(p0["Vp"])

        prev = p0

        # ---- steady pairs: front(p) + back(p-1) + next pair's Q/K chains
        # hoisted into the late scores gaps, so scores(p+1) can fire the
        # moment exp7(p) releases the scores PSUM ----
        cur = pair_tiles(1)
        proj_half(cur["QT"][0], "q", xTf, 1, 0)
        proj_half(cur["QT"][1], "q", xTf, 1, 1)
        proj_half(cur["KT2"][0], "k", xTt, 1, 0)
        proj_half(cur["KT2"][1], "k", xTt, 1, 1)

        for hp in range(1, NP):
            mt = hp
            nxt = pair_tiles(mt + 1) if hp < NP - 1 else None

            scores_jt(cur["QT"], cur["KT2"], 0, cur["Et"])
            ctx_chunk(prev, 0, 0)
            scores_jt(cur["QT"], cur["KT2"], 1, cur["Et"])
            proj_half(cur["VT2"][0], "v", xTt, mt, 0)
            scores_jt(cur["QT"], cur["KT2"], 2, cur["Et"])
            ctx_chunk(prev, 0, 1)
            scores_jt(cur["QT"], cur["KT2"], 3, cur["Et"])
            outT_norm(prev, 0)
            scores_jt(cur["QT"], cur["KT2"], 4, cur["Et"])
            ctx_chunk(prev, 1, 0)
            scores_jt(cur["QT"], cur["KT2"], 5, cur["Et"])
            proj_half(cur["VT2"][1], "v", xTt, mt, 1)
            ctx_chunk(prev, 1, 1)
            scores_jt(cur["QT"], cur["KT2"], 6, cur["Et"])
            if nxt is not None:
                proj_half(nxt["QT"][0], "q", xTf, mt + 1, 0)
                proj_half(nxt["QT"][1], "q", xTf, mt + 1, 1)
                proj_half(nxt["KT2"][0], "k", xTt, mt + 1, 0)
            scores_jt(cur["QT"], cur["KT2"], 7, cur["Et"])
            outT_norm(prev, 1)
            if nxt is not None:
                proj_half(nxt["KT2"][1], "k", xTt, mt + 1, 1)
            vprime_strips(cur["VT2"], cur["Vp"], range(0, 4))
            vprime_strips(cur["VT2"], cur["Vp"], range(4, 8))
            vprime_ones(cur["Vp"])
            out_dma(prev)

            if nxt is not None:
                prev, cur = cur, nxt
            else:
                prev = cur

        # ---- drain: back(7); both hh0/hh1 first chunks are emitted up
        # front so their jt0-6 matmuls prefix-run before the last exp lands
        ctx_chunk(prev, 0, 0)
        ctx_chunk(prev, 1, 0)
        ctx_chunk(prev, 0, 1)
        outT_norm(prev, 0)
        ctx_chunk(prev, 1, 1)
        outT_norm(prev, 1)
        out_dma(prev)

    nc.compile()
    return nc


def run(inputs, trace=False, trace_kwargs=None):
    """inputs: dict of full-shape np arrays as in reference.setup_inputs()."""
    nc = build_kernel()
    in_maps = []
    for b in range(N_CORES):
        in_maps.append({
            "from_tensor": np.ascontiguousarray(np.asarray(inputs["from_tensor"][b], dtype=np.float32)),
            "to_tensor": np.ascontiguousarray(np.asarray(inputs["to_tensor"][b], dtype=np.float32)),
            "Wq": np.asarray(inputs["Wq"], dtype=np.float32),
            "bq": np.asarray(inputs["bq"], dtype=np.float32),
            "Wk": np.asarray(inputs["Wk"], dtype=np.float32),
            "bk": np.asarray(inputs["bk"], dtype=np.float32),
            "Wv": np.asarray(inputs["Wv"], dtype=np.float32),
            "bv": np.asarray(inputs["bv"], dtype=np.float32),
        })
    res = run_bass_kernel_spmd(nc, in_maps, core_ids=list(range(N_CORES)),
                               trace=trace, **(trace_kwargs or {}))
    out = np.stack([np.asarray(res.results[b]["out"]) for b in range(N_CORES)],
                   axis=0).astype(np.float32)
    return out, res


def kernel(**inputs):
    out, _ = run(inputs, trace=False)
    return out

